# revision 1
# baseline (speedup 1.0000x reference)
"""Trainium2 Bass kernel for ContPeepholeLSTMFunc_Delay.

Strategy (pure data parallel, per spec sharding hint):
  - Shard batch B=32768 across 8 cores (4096 rows/core).
  - On-chip layout is feature-major ([feature_chunk(128 part), batch(free)]),
    produced by host-side transposes, so matmuls need no on-chip transposes:
    out[m=h_out, n=batch] = lhsT.T @ rhs with lhsT = W.T chunk (stationary)
    and rhs = activation chunk (moving).
  - Matmul operands in bf16 (fp32 PSUM accumulation); elementwise in fp32.
  - Per core: 8 batch blocks of 512 columns; per block, phase 1 computes
    i/f/c_tilde gates + c_t + dc_dt, phase 2 computes o-gate/dC (which need
    full c_t / dc_dt across all four 128-row h-chunks) and the final dh_dt.
  - Biases are folded into the gate activations via the ACT bias operand
    (b_i = bUi + bPi etc.); (Ui+Pi) and (Uf+Pf) are precombined on host.
"""

import numpy as np
import ml_dtypes

import concourse.bacc as bacc
import concourse.mybir as mybir
import concourse.tile as tile
from concourse.bass_utils import run_bass_kernel_spmd

B, I, H = 32768, 256, 512
NCORES = 8
BC = B // NCORES            # 4096 rows per core
NBLK = 8                    # batch blocks per core
NB = BC // NBLK             # 512 batch columns per block
KI = I // 128               # 2 k-chunks for x-side matmuls
KH = H // 128               # 4 k-chunks for h-side matmuls

F32 = mybir.dt.float32
BF16 = mybir.dt.bfloat16
AF = mybir.ActivationFunctionType
OP = mybir.AluOpType
BF16_NP = ml_dtypes.bfloat16


def _mm_acc_open(nc, psum_tile, groups, mo, start, stop=False):
    """Emit a partial accumulation segment into psum_tile."""
    total = sum(g[2] for g in groups)
    idx = 0
    ms = slice(mo * 128, (mo + 1) * 128)
    for w_sb, act_sb, kk in groups:
        for k in range(kk):
            nc.tensor.matmul(
                psum_tile[:],
                w_sb[:, k, ms],
                act_sb[:, k, :],
                start=(start and idx == 0),
                stop=(stop and idx == total - 1),
            )
            idx += 1


def _mm_acc(nc, psum_tile, groups, mo):
    """Accumulate sum_g act_g @ W_g into psum_tile for output chunk mo.

    groups: list of (w_sbuf [128, K, H], act_sbuf [128, K, NB], K).
    """
    total = sum(g[2] for g in groups)
    idx = 0
    ms = slice(mo * 128, (mo + 1) * 128)
    for w_sb, act_sb, kk in groups:
        for k in range(kk):
            nc.tensor.matmul(
                psum_tile[:],
                w_sb[:, k, ms],
                act_sb[:, k, :],
                start=(idx == 0),
                stop=(idx == total - 1),
            )
            idx += 1


def build_nc(compile=True, opt=None):
    """opt flags: 'worder' weight-DMA reorder, 'cbfsplit' per-chunk c_past
    downcast, 'ps62' 6/2 psum tag split, 'ph2seg' two-segment ph2 groups."""
    if opt is None:
        opt = {"worder", "cbfsplit"}
    nc = bacc.Bacc(None, target_bir_lowering=False)

    # packed bf16 activations: x(0:2) | h(2:6) | cg(6:8) | dh(8:12) on axis 2
    pk_d = nc.dram_tensor("pkT", [NBLK, 128, 2 * KI + 2 * KH, NB], BF16, kind="ExternalInput")
    # packed f32 activations: c(0:4) | dc(4:8)
    pf_d = nc.dram_tensor("pfT", [NBLK, 128, 2 * KH, NB], F32, kind="ExternalInput")

    w_shapes = {}
    for nm in ("wi", "wf", "wo", "wc"):
        w_shapes[nm] = [128, KI, H]
    for nm in ("ui", "uf", "uc", "uo", "pi", "pf", "po", "upi", "upf"):
        w_shapes[nm] = [128, KH, H]
    w_d = {
        nm: nc.dram_tensor(nm, shp, BF16, kind="ExternalInput")
        for nm, shp in w_shapes.items()
    }
    # packed biases: bi | bf | bc | bo, each KH columns
    b_d = nc.dram_tensor("biases", [128, 4 * KH], F32, kind="ExternalInput")

    out_d = nc.dram_tensor("outT", [NBLK, KH, 128, NB], F32, kind="ExternalOutput")

    with tile.TileContext(nc) as tc:
        with (
            tc.tile_pool(name="wpool", bufs=1) as wp,
            tc.tile_pool(name="inp", bufs=2) as inp,
            tc.tile_pool(name="blk", bufs=2) as blkp,
            tc.tile_pool(name="scr", bufs=16) as scr,
            tc.tile_pool(name="psum", bufs=8, space="PSUM") as pp,
        ):
            W = {}

            def load_w(names):
                for nm in names:
                    W[nm] = wp.tile(w_shapes[nm], BF16, tag=nm, name=f"w_{nm}")
                    nc.sync.dma_start(W[nm][:], w_d[nm][:])

            # critical set for the very first matmul group; the rest of the
            # weights are loaded after block 0's inputs (ordered by first use)
            b_sb = wp.tile([128, 4 * KH], F32, tag="biases", name="b_sb")
            nc.sync.dma_start(b_sb[:], b_d[:])
            load_w(("wi", "ui", "pi"))
            Bs = {
                "bi": b_sb[:, 0 * KH : 1 * KH],
                "bf": b_sb[:, 1 * KH : 2 * KH],
                "bc": b_sb[:, 2 * KH : 3 * KH],
                "bo": b_sb[:, 3 * KH : 4 * KH],
            }

            for nb in range(NBLK):
                NPK = 2 * KI + 2 * KH
                pk_sb = inp.tile([128, NPK, NB], BF16, tag="pk", name="pk_sb")
                # x|h half first (feeds the forward gate matmuls), cg|dh second
                nc.sync.dma_start(pk_sb[:, 0 : NPK // 2], pk_d[nb, :, 0 : NPK // 2])
                pf_sb = inp.tile([128, 2 * KH, NB], F32, tag="pf", name="pf_sb")
                # c_past half right after x|h (feeds cbf + the Pi/Pf matmuls)
                nc.sync.dma_start(pf_sb[:, 0:KH], pf_d[nb, :, 0:KH])
                nc.sync.dma_start(pk_sb[:, NPK // 2 : NPK], pk_d[nb, :, NPK // 2 : NPK])
                nc.sync.dma_start(pf_sb[:, KH : 2 * KH], pf_d[nb, :, KH : 2 * KH])
                x_sb = pk_sb[:, 0:KI]
                h_sb = pk_sb[:, KI : KI + KH]
                cg_sb = pk_sb[:, KI + KH : 2 * KI + KH]
                dh_sb = pk_sb[:, 2 * KI + KH : 2 * KI + 2 * KH]
                c_sb = pf_sb[:, 0:KH]
                dc_sb = pf_sb[:, KH : 2 * KH]

                if nb == 0:
                    # remaining weights, ordered by first use, behind block-0 inputs
                    load_w(("wf", "uf", "pf", "wc", "uc", "upi", "upf", "wo", "uo", "po"))

                # bf16 copy of c_past for the Pi/Pf matmul operand
                cbf = inp.tile([128, KH, NB], BF16, tag="cbf", name="cbf")
                if "cbfsplit" in opt:
                    for k in range(KH):
                        nc.scalar.copy(cbf[:, k, :], c_sb[:, k, :])
                else:
                    nc.scalar.copy(cbf[:], c_sb[:])

                tanhc = blkp.tile([128, KH, NB], F32, tag="tanhc", name="tanhc")
                dcdt = blkp.tile([128, KH, NB], F32, tag="dcdt", name="dcdt")
                ctbf = blkp.tile([128, KH, NB], BF16, tag="ctbf", name="ctbf")
                dcbf = blkp.tile([128, KH, NB], BF16, tag="dcbf", name="dcbf")

                if "ps62" in opt:
                    tag1, bufs1, tag2, bufs2 = "ps1", 6, "ps2", 2
                else:
                    tag1, bufs1, tag2, bufs2 = "ps", 8, "ps", 8

                # ---------------- phase 1: i, f, c_tilde, c_t, dc_dt ----------
                ph1_defs = [
                    ("ps_i", [(W["wi"], x_sb, KI), (W["ui"], h_sb, KH), (W["pi"], cbf, KH)]),
                    ("ps_f", [(W["wf"], x_sb, KI), (W["uf"], h_sb, KH), (W["pf"], cbf, KH)]),
                    ("ps_c", [(W["wc"], x_sb, KI), (W["uc"], h_sb, KH)]),
                    ("pa", [(W["wi"], cg_sb, KI), (W["upi"], dh_sb, KH)]),
                    ("pb", [(W["wf"], cg_sb, KI), (W["upf"], dh_sb, KH)]),
                    ("pd", [(W["wc"], cg_sb, KI), (W["uc"], dh_sb, KH)]),
                ]
                for mo in range(KH):
                    ph1_ps = {}
                    for qname, groups in ph1_defs:
                        t = pp.tile([128, NB], F32, tag=tag1, bufs=bufs1, name=qname)
                        _mm_acc(nc, t, groups, mo)
                        ph1_ps[qname] = t
                    ps_i, ps_f, ps_c, pa, pb, pd = (
                        ph1_ps[q] for q in ("ps_i", "ps_f", "ps_c", "pa", "pb", "pd"))

                    i_t = scr.tile([128, NB], F32, tag="scr", name="i_t")
                    nc.scalar.activation(i_t[:], ps_i[:], AF.Sigmoid, bias=Bs["bi"][:, mo : mo + 1])
                    f_t = scr.tile([128, NB], F32, tag="scr", name="f_t")
                    nc.scalar.activation(f_t[:], ps_f[:], AF.Sigmoid, bias=Bs["bf"][:, mo : mo + 1])
                    c_til = scr.tile([128, NB], F32, tag="scr", name="c_til")
                    nc.scalar.activation(c_til[:], ps_c[:], AF.Tanh, bias=Bs["bc"][:, mo : mo + 1])

                    # c_t = f*c + i*c_tilde, written directly as bf16 (the Po
                    # matmul operand); tanh reads the bf16 c_t (err ~5e-4)
                    tfc = scr.tile([128, NB], F32, tag="scr", name="tfc")
                    nc.gpsimd.tensor_mul(tfc[:], f_t[:], c_sb[:, mo, :])
                    mic = scr.tile([128, NB], F32, tag="scr", name="mic")
                    nc.vector.tensor_mul(mic[:], i_t[:], c_til[:])
                    nc.vector.tensor_add(ctbf[:, mo, :], mic[:], tfc[:])
                    nc.scalar.activation(tanhc[:, mo, :], ctbf[:, mo, :], AF.Tanh)

                    # didt = i*(1-i)*dA   via ni = (i-1)*i ; didt = (pa*-1)*ni
                    ni = scr.tile([128, NB], F32, tag="scr", name="ni")
                    nc.vector.scalar_tensor_tensor(ni[:], i_t[:], 1.0, i_t[:], OP.subtract, OP.mult)
                    didt = scr.tile([128, NB], F32, tag="scr", name="didt")
                    nc.vector.scalar_tensor_tensor(didt[:], pa[:], -1.0, ni[:], OP.mult, OP.mult)
                    nf = scr.tile([128, NB], F32, tag="scr", name="nf")
                    nc.vector.scalar_tensor_tensor(nf[:], f_t[:], 1.0, f_t[:], OP.subtract, OP.mult)
                    dfdt = scr.tile([128, NB], F32, tag="scr", name="dfdt")
                    nc.vector.scalar_tensor_tensor(dfdt[:], pb[:], -1.0, nf[:], OP.mult, OP.mult)

                    # ndctil = (c_til^2 - 1)*dD  == -dc_tilde_dt
                    sqc = scr.tile([128, NB], F32, tag="scr", name="sqc")
                    nc.scalar.activation(sqc[:], c_til[:], AF.Square)
                    ndctil = scr.tile([128, NB], F32, tag="scr", name="ndctil")
                    nc.vector.scalar_tensor_tensor(ndctil[:], sqc[:], 1.0, pd[:], OP.subtract, OP.mult)

                    # dc_dt = dfdt*c + f*dcp + didt*c_til - i*ndctil
                    a1 = scr.tile([128, NB], F32, tag="scr", name="a1")
                    nc.vector.tensor_mul(a1[:], dfdt[:], c_sb[:, mo, :])
                    a2 = scr.tile([128, NB], F32, tag="scr", name="a2")
                    nc.gpsimd.tensor_mul(a2[:], f_t[:], dc_sb[:, mo, :])
                    a3 = scr.tile([128, NB], F32, tag="scr", name="a3")
                    nc.gpsimd.tensor_mul(a3[:], didt[:], c_til[:])
                    a4 = scr.tile([128, NB], F32, tag="scr", name="a4")
                    if nb == NBLK - 1:
                        # last block: keep the pd-critical tail off Pool's slow
                        # FIFO — nothing left to hide its latency behind
                        nc.vector.tensor_mul(a4[:], i_t[:], ndctil[:])
                    else:
                        nc.gpsimd.tensor_mul(a4[:], i_t[:], ndctil[:])
                    s1 = scr.tile([128, NB], F32, tag="scr", name="s1")
                    nc.vector.tensor_add(s1[:], a1[:], a2[:])
                    s2 = scr.tile([128, NB], F32, tag="scr", name="s2")
                    nc.vector.tensor_sub(s2[:], a3[:], a4[:])
                    nc.vector.tensor_add(dcdt[:, mo, :], s1[:], s2[:])
                    nc.scalar.copy(dcbf[:, mo, :], dcdt[:, mo, :])

                # ---------------- phase 2: o gate, dC, dh_dt ------------------
                for mo in range(KH):
                    ps_o = pp.tile([128, NB], F32, tag=tag2, bufs=bufs2, name="ps_o")
                    pC = pp.tile([128, NB], F32, tag=tag2, bufs=bufs2, name="pC")
                    if "ph2seg" in opt:
                        # independent x/h/cg/dh parts first; ctbf/dcbf-dependent
                        # Po parts last, so PE has fill work while the c_t /
                        # dc_dt elementwise chains finish.
                        _mm_acc_open(nc, ps_o, [(W["wo"], x_sb, KI), (W["uo"], h_sb, KH)], mo, start=True)
                        _mm_acc_open(nc, pC, [(W["wo"], cg_sb, KI), (W["uo"], dh_sb, KH)], mo, start=True)
                        _mm_acc_open(nc, ps_o, [(W["po"], ctbf, KH)], mo, start=False, stop=True)
                        _mm_acc_open(nc, pC, [(W["po"], dcbf, KH)], mo, start=False, stop=True)
                    else:
                        _mm_acc(nc, ps_o, [(W["wo"], x_sb, KI), (W["uo"], h_sb, KH), (W["po"], ctbf, KH)], mo)
                        _mm_acc(nc, pC, [(W["wo"], cg_sb, KI), (W["uo"], dh_sb, KH), (W["po"], dcbf, KH)], mo)

                    # off-critical-path pieces first: sqt/u only need ph1 results
                    sqt = scr.tile([128, NB], F32, tag="scr", name="sqt")
                    nc.scalar.activation(sqt[:], tanhc[:, mo, :], AF.Square)
                    u = scr.tile([128, NB], F32, tag="scr", name="u")
                    nc.vector.scalar_tensor_tensor(u[:], sqt[:], 1.0, dcdt[:, mo, :], OP.subtract, OP.mult)

                    o_t = scr.tile([128, NB], F32, tag="scr", name="o_t")
                    nc.scalar.activation(o_t[:], ps_o[:], AF.Sigmoid, bias=Bs["bo"][:, mo : mo + 1])
                    no_ = scr.tile([128, NB], F32, tag="scr", name="no_")
                    nc.vector.scalar_tensor_tensor(no_[:], o_t[:], 1.0, o_t[:], OP.subtract, OP.mult)
                    v = scr.tile([128, NB], F32, tag="scr", name="v")
                    nc.gpsimd.tensor_mul(v[:], o_t[:], u[:])
                    # m1 = -sig'_o * tanh_c precomputed so the pC-dependent tail
                    # is a single fused op: w_ = (pC * -1) * m1 = dodt * tanh_c
                    m1 = scr.tile([128, NB], F32, tag="scr", name="m1")
                    nc.vector.tensor_mul(m1[:], no_[:], tanhc[:, mo, :])

                    # dh = dodt*tanh_c + o*(1-tanh_c^2)*dc_dt
                    w_ = scr.tile([128, NB], F32, tag="scr", name="w_")
                    nc.vector.scalar_tensor_tensor(w_[:], pC[:], -1.0, m1[:], OP.mult, OP.mult)
                    ob = scr.tile([128, NB], F32, tag="scr", name="ob")
                    nc.vector.tensor_sub(ob[:], w_[:], v[:])
                    nc.sync.dma_start(out_d[nb, mo], ob[:])

    if compile:
        nc.compile()
    return nc


_NC_CACHE = None


def _get_nc():
    global _NC_CACHE
    if _NC_CACHE is None:
        _NC_CACHE = build_nc()
    return _NC_CACHE


def _prep_act(a, K, dtype):
    """[BC, K*128] (row-major batch) -> [NBLK, 128, K, NB] feature-major."""
    out = a.reshape(NBLK, NB, K, 128).transpose(0, 3, 2, 1)
    return np.ascontiguousarray(out).astype(dtype, copy=False)


def _fill_act(dst, a, K):
    """Write [BC, K*128] into dst ([NBLK, 128, K, NB]) transposed, in one pass
    (assignment fuses the strided gather with the dtype cast)."""
    dst[...] = a.reshape(NBLK, NB, K, 128).transpose(0, 3, 2, 1)


def _prep_w(w):
    """W [H_out, K_in] -> lhsT layout [128, K_in//128, H_out] bf16."""
    wt = np.asarray(w, np.float32).T  # [K_in, H_out]
    k = wt.shape[0] // 128
    out = wt.reshape(k, 128, wt.shape[1]).transpose(1, 0, 2)
    return np.ascontiguousarray(out).astype(BF16_NP)


def _prep_b(b):
    """[H] -> [128, KH] (partition-major per h-chunk)."""
    return np.ascontiguousarray(np.asarray(b, np.float32).reshape(KH, 128).T)


def _run(inputs, trace=False):
    nc = _get_nc()

    Wi, Wf, Wo, Wc = (np.asarray(inputs[k], np.float32) for k in ("Wi", "Wf", "Wo", "Wc"))
    Ui, Uf, Uo, Uc = (np.asarray(inputs[k], np.float32) for k in ("Ui", "Uf", "Uo", "Uc"))
    Pi, Pf, Po = (np.asarray(inputs[k], np.float32) for k in ("Pi", "Pf", "Po"))

    wmap = {
        "wi": _prep_w(Wi), "wf": _prep_w(Wf), "wo": _prep_w(Wo), "wc": _prep_w(Wc),
        "ui": _prep_w(Ui), "uf": _prep_w(Uf), "uc": _prep_w(Uc), "uo": _prep_w(Uo),
        "pi": _prep_w(Pi), "pf": _prep_w(Pf), "po": _prep_w(Po),
        "upi": _prep_w(Ui + Pi), "upf": _prep_w(Uf + Pf),
        "biases": np.ascontiguousarray(np.concatenate([
            _prep_b(np.asarray(inputs["bUi"]) + np.asarray(inputs["bPi"])),
            _prep_b(np.asarray(inputs["bUf"]) + np.asarray(inputs["bPf"])),
            _prep_b(np.asarray(inputs["bUc"])),
            _prep_b(np.asarray(inputs["bUo"]) + np.asarray(inputs["bPo"])),
        ], axis=1)),
    }

    x = np.asarray(inputs["x"], np.float32)
    cg = np.asarray(inputs["control_grad"], np.float32)
    h = np.asarray(inputs["h_past"], np.float32)
    dh = np.asarray(inputs["dhpast_dt"], np.float32)
    c = np.asarray(inputs["c_past"], np.float32)
    dc = np.asarray(inputs["dcpast_dt"], np.float32)

    in_maps = []
    for core in range(NCORES):
        sl = slice(core * BC, (core + 1) * BC)
        m = dict(wmap)
        pk = np.empty([NBLK, 128, 2 * KI + 2 * KH, NB], BF16_NP)
        _fill_act(pk[:, :, 0:KI], x[sl], KI)
        _fill_act(pk[:, :, KI : KI + KH], h[sl], KH)
        _fill_act(pk[:, :, KI + KH : 2 * KI + KH], cg[sl], KI)
        _fill_act(pk[:, :, 2 * KI + KH : 2 * KI + 2 * KH], dh[sl], KH)
        m["pkT"] = pk
        pf = np.empty([NBLK, 128, 2 * KH, NB], np.float32)
        _fill_act(pf[:, :, 0:KH], c[sl], KH)
        _fill_act(pf[:, :, KH : 2 * KH], dc[sl], KH)
        m["pfT"] = pf
        in_maps.append(m)

    try:
        res = run_bass_kernel_spmd(nc, in_maps, core_ids=list(range(NCORES)), trace=trace)
    except ModuleNotFoundError:
        if not trace:
            raise
        # NTFF profiling hook unavailable in this container — run untraced
        res = run_bass_kernel_spmd(nc, in_maps, core_ids=list(range(NCORES)), trace=False)

    outs = []
    for core in range(NCORES):
        o = res.results[core]["outT"]  # [NBLK, KH, 128, NB]
        o = o.transpose(0, 3, 1, 2).reshape(BC, H)
        outs.append(o)
    full = np.ascontiguousarray(np.concatenate(outs, axis=0), dtype=np.float32)
    return full, res


def kernel(**inputs):
    return _run(inputs, trace=False)[0]

